# revision 4
# baseline (speedup 1.0000x reference)
"""RIENet loss kernel (keypoint/KNN MSE + global-align Huber-min loss) on 8 trn2 cores.

Sharding: core ci -> (b = ci // 4, n-chunk j = ci % 4).  Each core holds the full
tgt[b] (M=8192 points) and a 2048-column chunk of src_transformed[b] (N axis),
computing the [8192 x 2048] block of squared distances
  Q = ||t||^2 + ||s||^2 - 2 t.s
as 64 m-tiles of [128, 2048] via K=13 bf16 matmuls (2-way bf16-split products
plus both norm splits folded into the contraction, so PSUM holds finished Q).

The device only converts Q to bf16 and ships it: per tile, ScalarE copies
PSUM->SBUF for the left SE_COLS columns while DVE converts the right columns
(TT min against +inf), keeping both consumer latencies under the ~1.7us
4-matmul PE group so the 2-deep PSUM ping-pong never stalls the PE.  The
full [128, 2048] bf16 tile streams to HBM (~33 MB/core at ~330 GB/s).

Host finishes everything else: row mins (over n) and column mins (over m)
via uint16-view bf16 min-reductions, cross-chunk/cross-partition combines,
Huber + sums in f64, and the tiny keypoint/KNN MSE terms.
"""

import os
import numpy as np


def _ensure_path():
    try:
        import concourse  # noqa: F401
    except ImportError:
        import sys
        for p in ("/opt/trn_rl_repo", "/root/.axon_site/_ro/trn_rl_repo"):
            if os.path.isdir(p) and p not in sys.path:
                sys.path.insert(0, p)


_ensure_path()

import ml_dtypes  # noqa: E402
import concourse.bass as bass  # noqa: E402
import concourse.bacc as bacc  # noqa: E402
import concourse.tile as tile  # noqa: E402
import concourse.mybir as mybir  # noqa: E402
from concourse.bass_utils import run_bass_kernel_spmd  # noqa: E402

F32 = mybir.dt.float32
BF16 = mybir.dt.bfloat16
AL = mybir.AluOpType
AF = mybir.ActivationFunctionType
BF16NP = np.dtype(ml_dtypes.bfloat16)

MARGIN = 0.1
B, KP, KNN, N, M = 2, 256, 32, 8192, 8192
NCORES = 8
NSHARDS = NCORES // B          # 4 n-chunks per batch element
CHUNK = N // NSHARDS           # 2048
MI = M // 128                  # 64 m-tiles
KROWS = 13
BIGB = 1.0e30
SE_COLS = 1280                 # columns converted by ScalarE (rest: DVE)

_CACHE = {}


def _build():
    nc = bacc.Bacc("TRN2", target_bir_lowering=False, debug=False,
                   num_devices=NCORES)

    tA_d = nc.dram_tensor("tA", [KROWS, M], BF16, kind="ExternalInput")
    sA_d = nc.dram_tensor("sA", [KROWS, CHUNK], BF16, kind="ExternalInput")

    q_o = nc.dram_tensor("qship", [MI, 128, CHUNK], BF16,
                         kind="ExternalOutput")

    with tile.TileContext(nc) as tc:
        with (
            tc.tile_pool(name="const", bufs=1) as const,
            tc.tile_pool(name="qb", bufs=6) as qbp,
        ):
            sA = const.tile([KROWS, CHUNK], BF16)
            tA = const.tile([KROWS, M], BF16)
            big = const.tile([128, CHUNK - SE_COLS], BF16)

            nc.sync.dma_start(out=sA[:], in_=sA_d[:])
            nc.sync.dma_start(out=tA[:, :1024], in_=tA_d[:, :1024])
            nc.sync.dma_start(out=tA[:, 1024:], in_=tA_d[:, 1024:])
            nc.gpsimd.memset(big[:], BIGB)

            with tc.tile_pool(name="psum_main", bufs=2, space="PSUM") as pm:
                for mi in range(MI):
                    pt = pm.tile([128, CHUNK], F32, tag="pt")
                    lhsT = tA[:, mi * 128:(mi + 1) * 128]
                    for nj in range(4):
                        nc.tensor.matmul(
                            pt[:, nj * 512:(nj + 1) * 512],
                            lhsT=lhsT,
                            rhs=sA[:, nj * 512:(nj + 1) * 512],
                            start=True, stop=True,
                        )
                    qb = qbp.tile([128, CHUNK], BF16, tag="qb")
                    nc.scalar.copy(out=qb[:, :SE_COLS], in_=pt[:, :SE_COLS])
                    nc.vector.tensor_tensor(qb[:, SE_COLS:], pt[:, SE_COLS:],
                                            big[:], AL.min)
                    nc.sync.dma_start(out=q_o[mi], in_=qb[:])

    nc.compile()
    return nc


def _get_nc():
    if "nc" not in _CACHE:
        _CACHE["nc"] = _build()
    return _CACHE["nc"]


def _split2(x):
    """Exact 2-way bf16 split of an fp32 array: x ~= h + m."""
    f = np.float32
    h = x.astype(BF16NP)
    m = (x - h.astype(f)).astype(BF16NP)
    return h, m


# Row layout (KROWS=13):
#   t side: rows 0-2 th, 3-5 th, 6-8 tm, 9-10 nt split, 11-12 ones
#   s side: rows 0-2 sh, 3-5 sm, 6-8 sh, 9-10 ones,     11-12 ns split
_T_DEST = {0: [0, 3], 1: [6]}
_S_DEST = {0: [0, 6], 1: [3]}


def _pack_rows(x, ns, width, t_side):
    """Build the [KROWS, width] bf16 operand for one side.

    x: [3, width] fp32 (already scaled by -2 for the t side)
    ns: [width] fp32 squared-norm row (own side's norm)
    """
    out = np.zeros((KROWS, width), dtype=BF16NP)
    h, m = _split2(x)
    dest = _T_DEST if t_side else _S_DEST
    for lvl, w in enumerate((h, m)):
        for base in dest[lvl]:
            out[base:base + 3, :] = w
    nh, nm = _split2(ns)
    ones = np.ones((2, width), dtype=BF16NP)
    if t_side:
        out[9, :] = nh
        out[10, :] = nm
        out[11:13, :] = ones
    else:
        out[9:11, :] = ones
        out[11, :] = nh
        out[12, :] = nm
    return out


def _prepare_in_maps(src_keypoints, tgt_keypoints, rotation_ab, translation_ab,
                     src_keypoints_knn, tgt_keypoints_knn, src_transformed, tgt):
    f = np.float32
    st = np.ascontiguousarray(np.asarray(src_transformed, dtype=f))
    tg = np.ascontiguousarray(np.asarray(tgt, dtype=f))
    skp = np.asarray(src_keypoints, dtype=f)
    tkp = np.asarray(tgt_keypoints, dtype=f)
    rot = np.asarray(rotation_ab, dtype=f)
    tra = np.asarray(translation_ab, dtype=f)
    sknn = np.asarray(src_keypoints_knn, dtype=f)
    tknn = np.asarray(tgt_keypoints_knn, dtype=f)

    # keypoint / knn MSE losses: 0.04% of the FLOPs, computed host-side
    transformed = np.einsum("bij,bjk->bik", rot, skp) + tra[:, :, None]
    kp_loss = np.float64(((transformed - tkp) ** 2).sum()) / B
    knn_loss = np.float64(((sknn - tknn) ** 2).sum()) / (B * KNN)
    ncl = kp_loss + knn_loss

    tA_b = []
    for b in range(B):
        t = tg[b]                                   # [3, M]
        nt = (t * t).sum(axis=0)                    # [M]
        tA_b.append(_pack_rows(-2.0 * t, nt, M, True))
    in_maps = []
    for ci in range(NCORES):
        b, j = divmod(ci, NSHARDS)
        s = np.ascontiguousarray(st[b, :, j * CHUNK:(j + 1) * CHUNK])
        ns = (s * s).sum(axis=0)
        in_maps.append({"tA": tA_b[b], "sA": _pack_rows(s, ns, CHUNK, False)})
    return in_maps, ncl


def _huber(x, c):
    return np.where(x < c, 0.5 * x * x, c * x - 0.5 * c * c)


def _postprocess(results):
    c = np.float64(MARGIN)
    loss1 = np.float64(0.0)
    loss2 = np.float64(0.0)
    for b in range(B):
        rowmin_b = None
        for j in range(NSHARDS):
            r = results[b * NSHARDS + j]
            q = np.asarray(r["qship"])               # [MI, 128, CHUNK]
            u = q.view(np.uint16)
            if (u & 0x8000).any():
                qf = q.astype(np.float32)
                colmin = qf.min(axis=(0, 1)).astype(np.float64)
                rm = qf.min(axis=2)                  # [MI, 128]
            else:
                colmin = u.min(axis=(0, 1)).view(BF16NP).astype(np.float64)
                rm = u.min(axis=2).view(BF16NP).astype(np.float32)
            loss1 += _huber(colmin, c).sum()
            rm = rm.reshape(M)
            rowmin_b = rm if rowmin_b is None else np.minimum(rowmin_b, rm)
        loss2 += _huber(rowmin_b.astype(np.float64), c).sum()
    gal = loss1 + loss2
    return np.float32(gal)


def run_device(in_maps, **kw):
    nc = _get_nc()
    return run_bass_kernel_spmd(nc, in_maps, list(range(NCORES)), **kw)


def kernel(src_keypoints, tgt_keypoints, rotation_ab, translation_ab,
           src_keypoints_knn, tgt_keypoints_knn, k, src_transformed, tgt,
           **_unused):
    in_maps, ncl = _prepare_in_maps(src_keypoints, tgt_keypoints, rotation_ab,
                                    translation_ab, src_keypoints_knn,
                                    tgt_keypoints_knn, src_transformed, tgt)
    res = run_device(in_maps)
    gal = _postprocess(res.results)
    return np.float32(ncl), gal


# revision 5
# speedup vs baseline: 1.0041x; 1.0041x over previous
"""RIENet loss kernel (keypoint/KNN MSE + global-align Huber-min loss) on 8 trn2 cores.

Sharding: core ci -> (b = ci // 4, n-chunk j = ci % 4).  Each core holds the full
tgt[b] (M=8192 points) and a 2048-column chunk of src_transformed[b] (N axis),
computing the [8192 x 2048] block of squared distances
  Q = ||t||^2 + ||s||^2 - 2 t.s
as 64 m-tiles of [128, 2048] via K=13 bf16 matmuls (2-way bf16-split products
plus both norm splits folded into the contraction, so PSUM holds finished Q).

The device only converts Q to bf16 and ships it: per tile, ScalarE copies
PSUM->SBUF for the left SE_COLS columns while DVE converts the right columns
(TT min against +inf), keeping both consumer latencies under the ~1.7us
4-matmul PE group so the 2-deep PSUM ping-pong never stalls the PE.  The
full [128, 2048] bf16 tile streams to HBM (~33 MB/core at ~330 GB/s).

Host finishes everything else: row mins (over n) and column mins (over m)
via uint16-view bf16 min-reductions, cross-chunk/cross-partition combines,
Huber + sums in f64, and the tiny keypoint/KNN MSE terms.
"""

import os
import numpy as np


def _ensure_path():
    try:
        import concourse  # noqa: F401
    except ImportError:
        import sys
        for p in ("/opt/trn_rl_repo", "/root/.axon_site/_ro/trn_rl_repo"):
            if os.path.isdir(p) and p not in sys.path:
                sys.path.insert(0, p)


_ensure_path()

import ml_dtypes  # noqa: E402
import concourse.bass as bass  # noqa: E402
import concourse.bacc as bacc  # noqa: E402
import concourse.tile as tile  # noqa: E402
import concourse.mybir as mybir  # noqa: E402
from concourse.bass_utils import run_bass_kernel_spmd  # noqa: E402

F32 = mybir.dt.float32
BF16 = mybir.dt.bfloat16
AL = mybir.AluOpType
AF = mybir.ActivationFunctionType
BF16NP = np.dtype(ml_dtypes.bfloat16)

MARGIN = 0.1
B, KP, KNN, N, M = 2, 256, 32, 8192, 8192
NCORES = 8
NSHARDS = NCORES // B          # 4 n-chunks per batch element
CHUNK = N // NSHARDS           # 2048
MI = M // 128                  # 64 m-tiles
KROWS = 13
BIGB = 1.0e30
SE_COLS = 1280                 # columns converted by ScalarE (rest: DVE)

_CACHE = {}


def _build():
    nc = bacc.Bacc("TRN2", target_bir_lowering=False, debug=False,
                   num_devices=NCORES)

    tA_d = nc.dram_tensor("tA", [KROWS, M], BF16, kind="ExternalInput")
    sA_d = nc.dram_tensor("sA", [KROWS, CHUNK], BF16, kind="ExternalInput")

    q_o = nc.dram_tensor("qship", [MI, 128, CHUNK], BF16,
                         kind="ExternalOutput")

    with tile.TileContext(nc) as tc:
        with (
            tc.tile_pool(name="const", bufs=1) as const,
            tc.tile_pool(name="qbl", bufs=6) as qlp,
            tc.tile_pool(name="qbr", bufs=6) as qrp,
        ):
            sA = const.tile([KROWS, CHUNK], BF16)
            tA = const.tile([KROWS, M], BF16)
            big = const.tile([128, CHUNK - SE_COLS], BF16)

            nc.sync.dma_start(out=sA[:], in_=sA_d[:])
            TCH = 1024
            for c in range(M // TCH):
                nc.sync.dma_start(out=tA[:, c * TCH:(c + 1) * TCH],
                                  in_=tA_d[:, c * TCH:(c + 1) * TCH])
            nc.gpsimd.memset(big[:], BIGB)

            with tc.tile_pool(name="psum_main", bufs=2, space="PSUM") as pm:
                for mi in range(MI):
                    pt = pm.tile([128, CHUNK], F32, tag="pt")
                    lhsT = tA[:, mi * 128:(mi + 1) * 128]
                    for nj in range(4):
                        nc.tensor.matmul(
                            pt[:, nj * 512:(nj + 1) * 512],
                            lhsT=lhsT,
                            rhs=sA[:, nj * 512:(nj + 1) * 512],
                            start=True, stop=True,
                        )
                    qbl = qlp.tile([128, SE_COLS], BF16, tag="qbl")
                    qbr = qrp.tile([128, CHUNK - SE_COLS], BF16, tag="qbr")
                    nc.scalar.copy(out=qbl[:], in_=pt[:, :SE_COLS])
                    nc.vector.tensor_tensor(qbr[:], pt[:, SE_COLS:],
                                            big[:], AL.min)
                    nc.sync.dma_start(out=q_o[mi, :, :SE_COLS], in_=qbl[:])
                    nc.sync.dma_start(out=q_o[mi, :, SE_COLS:], in_=qbr[:])

    nc.compile()
    return nc


def _get_nc():
    if "nc" not in _CACHE:
        _CACHE["nc"] = _build()
    return _CACHE["nc"]


def _split2(x):
    """Exact 2-way bf16 split of an fp32 array: x ~= h + m."""
    f = np.float32
    h = x.astype(BF16NP)
    m = (x - h.astype(f)).astype(BF16NP)
    return h, m


# Row layout (KROWS=13):
#   t side: rows 0-2 th, 3-5 th, 6-8 tm, 9-10 nt split, 11-12 ones
#   s side: rows 0-2 sh, 3-5 sm, 6-8 sh, 9-10 ones,     11-12 ns split
_T_DEST = {0: [0, 3], 1: [6]}
_S_DEST = {0: [0, 6], 1: [3]}


def _pack_rows(x, ns, width, t_side):
    """Build the [KROWS, width] bf16 operand for one side.

    x: [3, width] fp32 (already scaled by -2 for the t side)
    ns: [width] fp32 squared-norm row (own side's norm)
    """
    out = np.zeros((KROWS, width), dtype=BF16NP)
    h, m = _split2(x)
    dest = _T_DEST if t_side else _S_DEST
    for lvl, w in enumerate((h, m)):
        for base in dest[lvl]:
            out[base:base + 3, :] = w
    nh, nm = _split2(ns)
    ones = np.ones((2, width), dtype=BF16NP)
    if t_side:
        out[9, :] = nh
        out[10, :] = nm
        out[11:13, :] = ones
    else:
        out[9:11, :] = ones
        out[11, :] = nh
        out[12, :] = nm
    return out


def _prepare_in_maps(src_keypoints, tgt_keypoints, rotation_ab, translation_ab,
                     src_keypoints_knn, tgt_keypoints_knn, src_transformed, tgt):
    f = np.float32
    st = np.ascontiguousarray(np.asarray(src_transformed, dtype=f))
    tg = np.ascontiguousarray(np.asarray(tgt, dtype=f))
    skp = np.asarray(src_keypoints, dtype=f)
    tkp = np.asarray(tgt_keypoints, dtype=f)
    rot = np.asarray(rotation_ab, dtype=f)
    tra = np.asarray(translation_ab, dtype=f)
    sknn = np.asarray(src_keypoints_knn, dtype=f)
    tknn = np.asarray(tgt_keypoints_knn, dtype=f)

    # keypoint / knn MSE losses: 0.04% of the FLOPs, computed host-side
    transformed = np.einsum("bij,bjk->bik", rot, skp) + tra[:, :, None]
    kp_loss = np.float64(((transformed - tkp) ** 2).sum()) / B
    knn_loss = np.float64(((sknn - tknn) ** 2).sum()) / (B * KNN)
    ncl = kp_loss + knn_loss

    tA_b = []
    for b in range(B):
        t = tg[b]                                   # [3, M]
        nt = (t * t).sum(axis=0)                    # [M]
        tA_b.append(_pack_rows(-2.0 * t, nt, M, True))
    in_maps = []
    for ci in range(NCORES):
        b, j = divmod(ci, NSHARDS)
        s = np.ascontiguousarray(st[b, :, j * CHUNK:(j + 1) * CHUNK])
        ns = (s * s).sum(axis=0)
        in_maps.append({"tA": tA_b[b], "sA": _pack_rows(s, ns, CHUNK, False)})
    return in_maps, ncl


def _huber(x, c):
    return np.where(x < c, 0.5 * x * x, c * x - 0.5 * c * c)


def _postprocess(results):
    c = np.float64(MARGIN)
    loss1 = np.float64(0.0)
    loss2 = np.float64(0.0)
    for b in range(B):
        rowmin_b = None
        for j in range(NSHARDS):
            r = results[b * NSHARDS + j]
            q = np.asarray(r["qship"])               # [MI, 128, CHUNK]
            u = q.view(np.uint16)
            if (u & 0x8000).any():
                qf = q.astype(np.float32)
                colmin = qf.min(axis=(0, 1)).astype(np.float64)
                rm = qf.min(axis=2)                  # [MI, 128]
            else:
                colmin = u.min(axis=(0, 1)).view(BF16NP).astype(np.float64)
                rm = u.min(axis=2).view(BF16NP).astype(np.float32)
            loss1 += _huber(colmin, c).sum()
            rm = rm.reshape(M)
            rowmin_b = rm if rowmin_b is None else np.minimum(rowmin_b, rm)
        loss2 += _huber(rowmin_b.astype(np.float64), c).sum()
    gal = loss1 + loss2
    return np.float32(gal)


def run_device(in_maps, **kw):
    nc = _get_nc()
    return run_bass_kernel_spmd(nc, in_maps, list(range(NCORES)), **kw)


def kernel(src_keypoints, tgt_keypoints, rotation_ab, translation_ab,
           src_keypoints_knn, tgt_keypoints_knn, k, src_transformed, tgt,
           **_unused):
    in_maps, ncl = _prepare_in_maps(src_keypoints, tgt_keypoints, rotation_ab,
                                    translation_ab, src_keypoints_knn,
                                    tgt_keypoints_knn, src_transformed, tgt)
    res = run_device(in_maps)
    gal = _postprocess(res.results)
    return np.float32(ncl), gal


# revision 7
# speedup vs baseline: 1.2431x; 1.2380x over previous
"""RIENet loss kernel (keypoint/KNN MSE + global-align Huber-min loss) on 8 trn2 cores.

Sharding: core ci -> (b = ci // 4, n-chunk j = ci % 4).  Each core holds the full
tgt[b] (M=8192 points) and a 2048-column chunk of src_transformed[b] (N axis),
computing the [8192 x 2048] block of squared distances
  Q = ||t||^2 + ||s||^2 - 2 t.s
as 64 m-tiles of [128, 2048] via K=13 bf16 matmuls (2-way bf16-split products
plus both norm splits folded into the contraction, so PSUM holds finished Q).

The device only converts Q to bf16 and ships it: per tile, ScalarE copies
PSUM->SBUF for the left SE_COLS columns while DVE converts the right columns
(TT min against +inf), keeping both consumer latencies under the ~1.7us
4-matmul PE group so the 2-deep PSUM ping-pong never stalls the PE.  The
full [128, 2048] bf16 tile streams to HBM (~33 MB/core at ~330 GB/s).

Host finishes everything else: row mins (over n) and column mins (over m)
via uint16-view bf16 min-reductions, cross-chunk/cross-partition combines,
Huber + sums in f64, and the tiny keypoint/KNN MSE terms.
"""

import os
import numpy as np


def _ensure_path():
    try:
        import concourse  # noqa: F401
    except ImportError:
        import sys
        for p in ("/opt/trn_rl_repo", "/root/.axon_site/_ro/trn_rl_repo"):
            if os.path.isdir(p) and p not in sys.path:
                sys.path.insert(0, p)


_ensure_path()

import ml_dtypes  # noqa: E402
import concourse.bass as bass  # noqa: E402
import concourse.bacc as bacc  # noqa: E402
import concourse.tile as tile  # noqa: E402
import concourse.mybir as mybir  # noqa: E402
from concourse.bass_utils import run_bass_kernel_spmd  # noqa: E402

F32 = mybir.dt.float32
BF16 = mybir.dt.bfloat16
AL = mybir.AluOpType
AF = mybir.ActivationFunctionType
BF16NP = np.dtype(ml_dtypes.bfloat16)

MARGIN = 0.1
B, KP, KNN, N, M = 2, 256, 32, 8192, 8192
NCORES = 8
NSHARDS = NCORES // B          # 4 n-chunks per batch element
CHUNK = N // NSHARDS           # 2048
MI = M // 128                  # 64 m-tiles
KROWS = 13
BIGB = 1.0e30
SE_COLS = 1024                 # columns converted by ScalarE (rest: DVE)

_CACHE = {}


def _build():
    nc = bacc.Bacc("TRN2", target_bir_lowering=False, debug=False,
                   num_devices=NCORES)

    tA_d = nc.dram_tensor("tA", [KROWS, M], BF16, kind="ExternalInput")
    sA_d = nc.dram_tensor("sA", [KROWS, CHUNK], BF16, kind="ExternalInput")

    q_o = nc.dram_tensor("qship", [MI, 128, CHUNK], BF16,
                         kind="ExternalOutput")

    with tile.TileContext(nc) as tc:
        with (
            tc.tile_pool(name="const", bufs=1) as const,
            tc.tile_pool(name="qbl", bufs=6) as qlp,
            tc.tile_pool(name="qbr", bufs=6) as qrp,
        ):
            sA = const.tile([KROWS, CHUNK], BF16)
            tA = const.tile([KROWS, M], BF16)
            big = const.tile([128, CHUNK - SE_COLS], BF16)

            nc.sync.dma_start(out=sA[:], in_=sA_d[:])
            TCH = 1024
            for c in range(M // TCH):
                nc.sync.dma_start(out=tA[:, c * TCH:(c + 1) * TCH],
                                  in_=tA_d[:, c * TCH:(c + 1) * TCH])
            nc.gpsimd.memset(big[:], BIGB)

            with (
                tc.tile_pool(name="psum_l", bufs=2, space="PSUM") as pml,
                tc.tile_pool(name="psum_r", bufs=2, space="PSUM") as pmr,
            ):
                for mi in range(MI):
                    ptl = pml.tile([128, SE_COLS], F32, tag="ptl")
                    ptr = pmr.tile([128, CHUNK - SE_COLS], F32, tag="ptr")
                    lhsT = tA[:, mi * 128:(mi + 1) * 128]
                    for nj in range(2):
                        nc.tensor.matmul(
                            ptl[:, nj * 512:(nj + 1) * 512],
                            lhsT=lhsT,
                            rhs=sA[:, nj * 512:(nj + 1) * 512],
                            start=True, stop=True,
                        )
                    for nj in range(2):
                        nc.tensor.matmul(
                            ptr[:, nj * 512:(nj + 1) * 512],
                            lhsT=lhsT,
                            rhs=sA[:, 1024 + nj * 512:1024 + (nj + 1) * 512],
                            start=True, stop=True,
                        )
                    qbl = qlp.tile([128, SE_COLS], BF16, tag="qbl")
                    qbr = qrp.tile([128, CHUNK - SE_COLS], BF16, tag="qbr")
                    nc.scalar.copy(out=qbl[:], in_=ptl[:])
                    nc.vector.tensor_tensor(qbr[:], ptr[:], big[:], AL.min)
                    nc.sync.dma_start(out=q_o[mi, :, :SE_COLS], in_=qbl[:])
                    nc.sync.dma_start(out=q_o[mi, :, SE_COLS:], in_=qbr[:])

    nc.compile()
    return nc


def _get_nc():
    if "nc" not in _CACHE:
        _CACHE["nc"] = _build()
    return _CACHE["nc"]


def _split2(x):
    """Exact 2-way bf16 split of an fp32 array: x ~= h + m."""
    f = np.float32
    h = x.astype(BF16NP)
    m = (x - h.astype(f)).astype(BF16NP)
    return h, m


# Row layout (KROWS=13):
#   t side: rows 0-2 th, 3-5 th, 6-8 tm, 9-10 nt split, 11-12 ones
#   s side: rows 0-2 sh, 3-5 sm, 6-8 sh, 9-10 ones,     11-12 ns split
_T_DEST = {0: [0, 3], 1: [6]}
_S_DEST = {0: [0, 6], 1: [3]}


def _pack_rows(x, ns, width, t_side):
    """Build the [KROWS, width] bf16 operand for one side.

    x: [3, width] fp32 (already scaled by -2 for the t side)
    ns: [width] fp32 squared-norm row (own side's norm)
    """
    out = np.zeros((KROWS, width), dtype=BF16NP)
    h, m = _split2(x)
    dest = _T_DEST if t_side else _S_DEST
    for lvl, w in enumerate((h, m)):
        for base in dest[lvl]:
            out[base:base + 3, :] = w
    nh, nm = _split2(ns)
    ones = np.ones((2, width), dtype=BF16NP)
    if t_side:
        out[9, :] = nh
        out[10, :] = nm
        out[11:13, :] = ones
    else:
        out[9:11, :] = ones
        out[11, :] = nh
        out[12, :] = nm
    return out


def _prepare_in_maps(src_keypoints, tgt_keypoints, rotation_ab, translation_ab,
                     src_keypoints_knn, tgt_keypoints_knn, src_transformed, tgt):
    f = np.float32
    st = np.ascontiguousarray(np.asarray(src_transformed, dtype=f))
    tg = np.ascontiguousarray(np.asarray(tgt, dtype=f))
    skp = np.asarray(src_keypoints, dtype=f)
    tkp = np.asarray(tgt_keypoints, dtype=f)
    rot = np.asarray(rotation_ab, dtype=f)
    tra = np.asarray(translation_ab, dtype=f)
    sknn = np.asarray(src_keypoints_knn, dtype=f)
    tknn = np.asarray(tgt_keypoints_knn, dtype=f)

    # keypoint / knn MSE losses: 0.04% of the FLOPs, computed host-side
    transformed = np.einsum("bij,bjk->bik", rot, skp) + tra[:, :, None]
    kp_loss = np.float64(((transformed - tkp) ** 2).sum()) / B
    knn_loss = np.float64(((sknn - tknn) ** 2).sum()) / (B * KNN)
    ncl = kp_loss + knn_loss

    tA_b = []
    for b in range(B):
        t = tg[b]                                   # [3, M]
        nt = (t * t).sum(axis=0)                    # [M]
        tA_b.append(_pack_rows(-2.0 * t, nt, M, True))
    in_maps = []
    for ci in range(NCORES):
        b, j = divmod(ci, NSHARDS)
        s = np.ascontiguousarray(st[b, :, j * CHUNK:(j + 1) * CHUNK])
        ns = (s * s).sum(axis=0)
        in_maps.append({"tA": tA_b[b], "sA": _pack_rows(s, ns, CHUNK, False)})
    return in_maps, ncl


def _huber(x, c):
    return np.where(x < c, 0.5 * x * x, c * x - 0.5 * c * c)


def _postprocess(results):
    c = np.float64(MARGIN)
    loss1 = np.float64(0.0)
    loss2 = np.float64(0.0)
    for b in range(B):
        rowmin_b = None
        for j in range(NSHARDS):
            r = results[b * NSHARDS + j]
            q = np.asarray(r["qship"])               # [MI, 128, CHUNK]
            u = q.view(np.uint16)
            if (u & 0x8000).any():
                qf = q.astype(np.float32)
                colmin = qf.min(axis=(0, 1)).astype(np.float64)
                rm = qf.min(axis=2)                  # [MI, 128]
            else:
                colmin = u.min(axis=(0, 1)).view(BF16NP).astype(np.float64)
                rm = u.min(axis=2).view(BF16NP).astype(np.float32)
            loss1 += _huber(colmin, c).sum()
            rm = rm.reshape(M)
            rowmin_b = rm if rowmin_b is None else np.minimum(rowmin_b, rm)
        loss2 += _huber(rowmin_b.astype(np.float64), c).sum()
    gal = loss1 + loss2
    return np.float32(gal)


def run_device(in_maps, **kw):
    nc = _get_nc()
    return run_bass_kernel_spmd(nc, in_maps, list(range(NCORES)), **kw)


def kernel(src_keypoints, tgt_keypoints, rotation_ab, translation_ab,
           src_keypoints_knn, tgt_keypoints_knn, k, src_transformed, tgt,
           **_unused):
    in_maps, ncl = _prepare_in_maps(src_keypoints, tgt_keypoints, rotation_ab,
                                    translation_ab, src_keypoints_knn,
                                    tgt_keypoints_knn, src_transformed, tgt)
    res = run_device(in_maps)
    gal = _postprocess(res.results)
    return np.float32(ncl), gal
